# revision 20
# baseline (speedup 1.0000x reference)
"""DGCNN (nn_DGCNN_Model) Trainium2 Bass kernel.

Pure data-parallel over batch B=8 across 8 NeuronCores (one sample per
core, SPMD — identical program, per-core inputs).

Per-sample pipeline (N=2048 points, K=20 neighbors):
  Each EdgeConv(C->O) is decomposed as
     h[n,k,o] = G[idx[n,k],o] + C2[n,o]
     out[n,o] = lrelu( max_k h[n,k,o] )          (BN scale folded, s>0)
  with G = X @ (s*w_a)^T  (N,O),  C2 = X @ (s*(w_b-w_a))^T + b  (N,O).
  kNN scores S[n,j] = 2<x_n,x_j> - |x_j|^2 (row-constant -|x_n|^2 dropped;
  it does not change per-row top-k). Top-20 per row via DVE max8 /
  max_index / match_replace (3 rounds of 8). Neighbor rows of G gathered
  from a DRAM table via indirect DMA (one per neighbor slot), then
  max-reduced on DVE.
"""
import sys

sys.path.insert(0, "/opt/trn_rl_repo")

import numpy as np

import concourse.bass as bass
import concourse.bacc as bacc
import concourse.mybir as mybir
import concourse.tile as tile
from concourse.bass import IndirectOffsetOnAxis
from concourse.bass_utils import run_bass_kernel_spmd
from concourse.masks import make_identity

P = 128
N = 2048
NT = N // P  # 16 row tiles
KNB = 20
NCORES = 8
F32 = mybir.dt.float32
U32 = mybir.dt.uint32
NEG_BIG = -1.0e30

# (C_in, O_out) for the four EdgeConvs
LAYERS = [(3, 64), (64, 64), (64, 128), (128, 256)]


def _emit_edge_conv(nc, pools, C, O, x_ap, wg, wc, bb, g_dram, write_out):
    """Emit one EdgeConv layer.

    x_ap:  AP (C, N) — input features, feature-major, base partition 0.
    wg/wc: AP (C, O) SBUF — projection weights.   bb: AP (1, O) — bias row.
    g_dram: DRAM tensor handle (N, O) — scratch table for gathers.
    write_out(t, y): store tile-t output (P, O) into next-layer features.
    """
    ones, neg1 = pools["ones"], pools["neg1"]
    psum_s, psum_ab = pools["psum_s"], pools["psum_ab"]
    pool = pools["sbuf"]

    # ---- phase A: squared norms + projections ----
    # chunked (512-col) so each chunk only depends on the producing tiles
    # of the previous layer -> cross-layer overlap.
    xsq = pool.tile([P, N], F32, tag="xa")
    x2 = pool.tile([P, N], F32, tag="xa")
    xx_sb = pool.tile([1, N], F32, tag="xx", bufs=1)
    for j in range(4):
        cs = slice(j * 512, (j + 1) * 512)
        nc.scalar.activation(out=xsq[:C, cs], in_=x_ap[:, cs],
                             func=mybir.ActivationFunctionType.Square)
        xx_ps = psum_ab.tile([1, 512], F32, tag="ab", name=f"xxps{j}")
        nc.tensor.matmul(out=xx_ps[:], lhsT=ones[:C, :1], rhs=xsq[:C, cs],
                         start=True, stop=True)
        nc.scalar.copy(out=xx_sb[:, cs], in_=xx_ps[:])
        nc.scalar.activation(out=x2[:C, cs], in_=x_ap[:, cs],
                             func=mybir.ActivationFunctionType.Copy, scale=2.0)

    c2full = pool.tile([P, NT * O], F32, tag="c2full", bufs=1)
    for t in range(NT):
        rs = slice(t * P, (t + 1) * P)
        os_ = slice(t * O, (t + 1) * O)
        gp = psum_ab.tile([P, O], F32, tag="ab")
        nc.tensor.matmul(out=gp[:], lhsT=x_ap[:, rs], rhs=wg, start=True, stop=True)
        gsb = pool.tile([P, O], F32, tag="gsb")
        nc.scalar.copy(out=gsb[:], in_=gp[:])
        nc.sync.dma_start(out=g_dram[t * P:(t + 1) * P, :], in_=gsb[:])
        cp = psum_ab.tile([P, O], F32, tag="ab")
        nc.tensor.matmul(out=cp[:], lhsT=x_ap[:, rs], rhs=wc, start=True, stop=False)
        nc.tensor.matmul(out=cp[:], lhsT=ones[:1, :], rhs=bb, start=False, stop=True)
        nc.scalar.copy(out=c2full[:, os_], in_=cp[:])

    # ---- phase B: per-tile knn + gather-max + epilogue ----
    def _emit_epilogue(t, tg):
        m = pool.tile([P, O], F32, tag="m")
        nc.vector.tensor_reduce(out=m[:], in_=tg[:].rearrange("p k o -> p o k"),
                                axis=mybir.AxisListType.X, op=mybir.AluOpType.max)
        t1 = pool.tile([P, O], F32, tag="t1")
        nc.vector.tensor_tensor(out=t1[:], in0=m[:], in1=c2full[:, t * O:(t + 1) * O],
                                op=mybir.AluOpType.add)
        y = pool.tile([P, O], F32, tag="y")
        nc.vector.scalar_tensor_tensor(out=y[:], in0=t1[:], scalar=0.2, in1=t1[:],
                                       op0=mybir.AluOpType.mult,
                                       op1=mybir.AluOpType.max)
        write_out(t, y)

    pending = []
    for t in range(NT):
        rs = slice(t * P, (t + 1) * P)
        # scores in two PSUM halves so the copy frees banks earlier
        sph = [psum_s.tile([P, N // 2], F32, tag="s", name=f"sp{t}_{h}")
               for h in range(2)]
        ssb = pool.tile([P, N], F32, tag="ssb", bufs=3)
        for j in range(4):
            h, sp = j // 2, sph[j // 2]
            cs = slice(j * 512, (j + 1) * 512)
            cl = slice((j % 2) * 512, (j % 2 + 1) * 512)
            nc.tensor.matmul(out=sp[:, cl], lhsT=x2[:C, rs], rhs=x_ap[:, cs],
                             start=True, stop=False)
            nc.tensor.matmul(out=sp[:, cl], lhsT=neg1[:1, :], rhs=xx_sb[:, cs],
                             start=False, stop=True)
        for h in range(2):
            nc.scalar.copy(out=ssb[:, h * (N // 2):(h + 1) * (N // 2)],
                           in_=sph[h][:])

        # topk rounds; gathers for each round issue as soon as its indices
        # exist (separate index tiles -> independent dependencies)
        vals = pool.tile([P, 24], F32, tag="vals")
        idxsA = pool.tile([P, 8], U32, tag="idxsA")
        idxsB = pool.tile([P, 8], U32, tag="idxsB")
        idxsC = pool.tile([P, 8], U32, tag="idxsC")
        tg = pool.tile([P, KNB, O], F32, tag="tg", bufs=3)
        w1 = pool.tile([P, N], F32, tag="wrk")

        def gathers(idxt, k0, k1):
            for k in range(k0, k1):
                nc.gpsimd.indirect_dma_start(
                    out=tg[:, k, :], out_offset=None, in_=g_dram[:, :],
                    in_offset=IndirectOffsetOnAxis(ap=idxt[:, k - k0:k - k0 + 1],
                                                   axis=0),
                )

        nc.vector.max(out=vals[:, 0:8], in_=ssb[:])
        nc.vector.max_index(out=idxsA[:], in_max=vals[:, 0:8], in_values=ssb[:])
        gathers(idxsA, 0, 8)
        nc.vector.match_replace(out=w1[:], in_to_replace=vals[:, 0:8],
                                in_values=ssb[:], imm_value=NEG_BIG)
        nc.vector.max(out=vals[:, 8:16], in_=w1[:])
        nc.vector.max_index(out=idxsB[:], in_max=vals[:, 8:16], in_values=w1[:])
        gathers(idxsB, 8, 16)
        nc.vector.match_replace(out=ssb[:], in_to_replace=vals[:, 8:16],
                                in_values=w1[:], imm_value=NEG_BIG)
        nc.vector.max(out=vals[:, 16:24], in_=ssb[:])
        nc.vector.max_index(out=idxsC[:], in_max=vals[:, 16:24], in_values=ssb[:])
        gathers(idxsC, 16, KNB)

        # software pipelining: the epilogue of the PREVIOUS tile is emitted
        # here so its gather-dependent reduce sits after this tile's topk in
        # the serial DVE stream (engines execute in order — emitting the
        # reduce right after its own gathers would stall DVE on the DMAs).
        if len(pending) == 2:
            _emit_epilogue(*pending.pop(0))
        pending.append((t, tg))
    for args in pending:
        _emit_epilogue(*args)



def build_program(repeat=None):
    nc = bacc.Bacc("TRN2", num_devices=NCORES, debug=False)

    # ---------------- I/O declarations ----------------
    x_in = nc.dram_tensor("x", (3, N), F32, kind="ExternalInput")
    wgt = {}
    for i, (C, O) in enumerate(LAYERS, start=1):
        wgt[f"wg{i}"] = nc.dram_tensor(f"wg{i}", (C, O), F32, kind="ExternalInput")
        wgt[f"wc{i}"] = nc.dram_tensor(f"wc{i}", (C, O), F32, kind="ExternalInput")
        wgt[f"bb{i}"] = nc.dram_tensor(f"bb{i}", (1, O), F32, kind="ExternalInput")
    w5r_d = nc.dram_tensor("w5r", (P, 5 * 1024), F32, kind="ExternalInput")
    b5r_d = nc.dram_tensor("b5r", (1, 1024), F32, kind="ExternalInput")
    f1r_d = nc.dram_tensor("f1r", (P, 16 * 512), F32, kind="ExternalInput")
    b6r_d = nc.dram_tensor("b6r", (1, 512), F32, kind="ExternalInput")
    f2r_d = nc.dram_tensor("f2r", (P, 4 * 256), F32, kind="ExternalInput")
    b7r_d = nc.dram_tensor("b7r", (1, 256), F32, kind="ExternalInput")
    f3r_d = nc.dram_tensor("f3r", (P, 2 * 40), F32, kind="ExternalInput")
    b8r_d = nc.dram_tensor("b8r", (1, 40), F32, kind="ExternalInput")
    out_d = nc.dram_tensor("out", (1, 40), F32, kind="ExternalOutput")

    g_drams = [nc.dram_tensor(f"gtab{i}", (N, O), F32)
               for i, (C, O) in enumerate(LAYERS, start=1)]

    with tile.TileContext(nc) as tc:
        with tc.tile_pool(name="glob", bufs=1) as gpool:
            # constants (full 128-partition tiles so base partitions align)
            ones = gpool.tile([P, P], F32, tag="ones")
            nc.vector.memset(ones[:], 1.0)
            neg1 = gpool.tile([P, P], F32, tag="neg1")
            nc.vector.memset(neg1[:], -1.0)
            identity = gpool.tile([P, P], F32, tag="identity")
            make_identity(nc, identity[:])

            # persistent transposed feature tensors (x1, x2, x3, x4a, x4b)
            TA = gpool.tile([P, N], F32, tag="TA")
            TB = gpool.tile([P, N], F32, tag="TB")
            T1 = gpool.tile([P, N], F32, tag="T1")
            T2 = gpool.tile([P, N], F32, tag="T2")
            T3 = gpool.tile([P, N], F32, tag="T3")

            # small per-layer weights (always 128-partition allocs)
            wsb = {}
            for i, (C, O) in enumerate(LAYERS, start=1):
                for nm, rows, cols in ((f"wg{i}", C, O), (f"wc{i}", C, O),
                                       (f"bb{i}", 1, O)):
                    tl = gpool.tile([P, cols], F32, tag=nm)
                    nc.sync.dma_start(out=tl[:rows, :], in_=wgt[nm][:, :])
                    wsb[nm] = tl
            x1sb = gpool.tile([P, N], F32, tag="x1sb")
            nc.sync.dma_start(out=x1sb[:3, :], in_=x_in[:, :])

            rep_cm = tc.For_i(0, repeat, 1) if repeat else None
            if rep_cm is not None:
                rep_cm.__enter__()

            # -------- edge conv layers --------
            with (
                tc.tile_pool(name="lay_sbuf", bufs=2) as lpool,
                tc.tile_pool(name="lay_psum_s", bufs=3, space="PSUM") as psum_s,
                tc.tile_pool(name="lay_psum_ab", bufs=1, space="PSUM") as psum_ab,
                tc.tile_pool(name="lay_psum_tr", bufs=1, space="PSUM") as psum_tr,
            ):
                pools = {"ones": ones, "neg1": neg1, "sbuf": lpool,
                         "psum_s": psum_s, "psum_ab": psum_ab}

                # destination map: layer idx -> list of (col0, ncols, T)
                outmaps = [
                    [(0, 64, TA)],
                    [(0, 64, TB)],
                    [(0, 128, T1)],
                    [(0, 128, T2), (128, 128, T3)],
                ]
                xin_aps = [x1sb[:3, :], TA[:64, :], TB[:64, :], T1[:, :]]

                for li, (C, O) in enumerate(LAYERS):
                    def write_out(t, y, _li=li):
                        for (c0, nco, Tdst) in outmaps[_li]:
                            tp = psum_tr.tile([P, P], F32, tag="tr")
                            nc.tensor.transpose(out=tp[:nco, :],
                                                in_=y[:, c0:c0 + nco],
                                                identity=identity[:])
                            nc.scalar.copy(out=Tdst[:nco, t * P:(t + 1) * P],
                                           in_=tp[:nco, :])
                    _emit_edge_conv(nc, pools, C, O, xin_aps[li],
                                    wsb[f"wg{li+1}"][:C, :], wsb[f"wc{li+1}"][:C, :],
                                    wsb[f"bb{li+1}"][:1, :], g_drams[li], write_out)

            # -------- conv5 + global pooling --------
            with (
                tc.tile_pool(name="c5_sbuf", bufs=2) as c5pool,
                tc.tile_pool(name="c5_psum_mean", bufs=1, space="PSUM") as psum_mean,
            ):
                # conv5 K-chunks: (features, w5r block, K)
                chunks = [(TA, 0, 64), (TB, 1, 64), (T1, 2, 128),
                          (T2, 3, 128), (T3, 4, 128)]
                w5r = c5pool.tile([P, 5 * 1024], F32, tag="w5r", bufs=1)
                nc.sync.dma_start(out=w5r[:], in_=w5r_d[:, :])
                b5r = c5pool.tile([P, 1024], F32, tag="b5r", bufs=1)
                nc.sync.dma_start(out=b5r[:1, :], in_=b5r_d[:, :])
                HM = c5pool.tile([P, 1024], F32, tag="HM", bufs=1)
                mean_ps = psum_mean.tile([1, 1024], F32, tag="mean")
                psum5_cm = tc.tile_pool(name="c5_psum", bufs=2, space="PSUM")
                psum5 = psum5_cm.__enter__()
                for t in range(NT):
                    rs = slice(t * P, (t + 1) * P)
                    hp = psum5.tile([P, 1024], F32, tag="hp")
                    for ci, (Tt, blk, kc) in enumerate(chunks):
                        for h in range(2):
                            ns = slice(h * 512, (h + 1) * 512)
                            nc.tensor.matmul(
                                out=hp[:, ns], lhsT=Tt[:kc, rs],
                                rhs=w5r[:kc, blk * 1024 + h * 512:
                                        blk * 1024 + (h + 1) * 512],
                                start=(ci == 0), stop=False)
                    for h in range(2):
                        ns = slice(h * 512, (h + 1) * 512)
                        nc.tensor.matmul(out=hp[:, ns], lhsT=ones[:1, :],
                                         rhs=b5r[:1, ns], start=False, stop=True)
                    hc = c5pool.tile([P, 1024], F32, tag="hc")
                    nc.scalar.copy(out=hc[:], in_=hp[:])
                    hs = c5pool.tile([P, 1024], F32, tag="hs")
                    nc.vector.scalar_tensor_tensor(
                        out=hs[:], in0=hc[:], scalar=0.2, in1=hc[:],
                        op0=mybir.AluOpType.mult, op1=mybir.AluOpType.max)
                    if t == 0:
                        nc.vector.tensor_copy(out=HM[:], in_=hs[:])
                    else:
                        nc.vector.tensor_tensor(out=HM[:], in0=HM[:], in1=hs[:],
                                                op=mybir.AluOpType.max)
                    for h in range(2):
                        ns = slice(h * 512, (h + 1) * 512)
                        nc.tensor.matmul(out=mean_ps[:, ns], lhsT=ones[:, :1],
                                         rhs=hs[:, ns], start=(t == 0),
                                         stop=(t == NT - 1))

                psum5_cm.__exit__(None, None, None)
                with tc.tile_pool(name="d_psum", bufs=2, space="PSUM") as psum_d:
                    mp = c5pool.tile([P, 8], F32, tag="mp")
                    mn = c5pool.tile([P, 8], F32, tag="mn")
                    for j in range(8):
                        tp = psum_d.tile([P, P], F32, tag="dtr")
                        nc.tensor.transpose(out=tp[:], in_=HM[:, j * P:(j + 1) * P],
                                            identity=identity[:])
                        nc.vector.tensor_reduce(out=mp[:, j:j + 1], in_=tp[:],
                                                axis=mybir.AxisListType.X,
                                                op=mybir.AluOpType.max)
                    mean_sb = c5pool.tile([P, 1024], F32, tag="mean_sb")
                    nc.scalar.copy(out=mean_sb[:1, :], in_=mean_ps[:])
                    for j in range(8):
                        cp = psum_d.tile([P, 1], F32, tag="dcol")
                        nc.tensor.matmul(out=cp[:], lhsT=mean_sb[:1, j * P:(j + 1) * P],
                                         rhs=ones[:1, :1], start=True, stop=True)
                        nc.scalar.copy(out=mn[:, j:j + 1], in_=cp[:])

                    # -------- FC head --------
                    f1r = c5pool.tile([P, 16 * 512], F32, tag="f1r", bufs=1)
                    nc.sync.dma_start(out=f1r[:], in_=f1r_d[:, :])
                    b6r = c5pool.tile([P, 512], F32, tag="b6r")
                    nc.sync.dma_start(out=b6r[:1, :], in_=b6r_d[:, :])
                    f2r = c5pool.tile([P, 4 * 256], F32, tag="f2r")
                    nc.sync.dma_start(out=f2r[:], in_=f2r_d[:, :])
                    b7r = c5pool.tile([P, 256], F32, tag="b7r")
                    nc.sync.dma_start(out=b7r[:1, :], in_=b7r_d[:, :])
                    f3r = c5pool.tile([P, 2 * 40], F32, tag="f3r")
                    nc.sync.dma_start(out=f3r[:], in_=f3r_d[:, :])
                    b8r = c5pool.tile([P, 40], F32, tag="b8r")
                    nc.sync.dma_start(out=b8r[:1, :], in_=b8r_d[:, :])

                    fp1 = psum_d.tile([1, 512], F32, tag="fc")
                    for j in range(8):
                        nc.tensor.matmul(out=fp1[:], lhsT=mp[:, j:j + 1],
                                         rhs=f1r[:, j * 512:(j + 1) * 512],
                                         start=(j == 0), stop=False)
                    for j in range(8):
                        nc.tensor.matmul(out=fp1[:], lhsT=mn[:, j:j + 1],
                                         rhs=f1r[:, (8 + j) * 512:(9 + j) * 512],
                                         start=False, stop=False)
                    nc.tensor.matmul(out=fp1[:], lhsT=ones[:1, :1], rhs=b6r[:1, :],
                                     start=False, stop=True)
                    r1 = c5pool.tile([P, 512], F32, tag="r1")
                    nc.scalar.activation(out=r1[:1, :], in_=fp1[:],
                                         func=mybir.ActivationFunctionType.Relu)
                    r1t = c5pool.tile([P, 4], F32, tag="r1t")
                    for j in range(4):
                        cp = psum_d.tile([P, 1], F32, tag="dcol")
                        nc.tensor.matmul(out=cp[:], lhsT=r1[:1, j * P:(j + 1) * P],
                                         rhs=ones[:1, :1], start=True, stop=True)
                        nc.scalar.copy(out=r1t[:, j:j + 1], in_=cp[:])

                    fp2 = psum_d.tile([1, 256], F32, tag="fc")
                    for j in range(4):
                        nc.tensor.matmul(out=fp2[:], lhsT=r1t[:, j:j + 1],
                                         rhs=f2r[:, j * 256:(j + 1) * 256],
                                         start=(j == 0), stop=False)
                    nc.tensor.matmul(out=fp2[:], lhsT=ones[:1, :1], rhs=b7r[:1, :],
                                     start=False, stop=True)
                    r2 = c5pool.tile([P, 256], F32, tag="r2")
                    nc.scalar.activation(out=r2[:1, :], in_=fp2[:],
                                         func=mybir.ActivationFunctionType.Relu)
                    r2t = c5pool.tile([P, 2], F32, tag="r2t")
                    for j in range(2):
                        cp = psum_d.tile([P, 1], F32, tag="dcol")
                        nc.tensor.matmul(out=cp[:], lhsT=r2[:1, j * P:(j + 1) * P],
                                         rhs=ones[:1, :1], start=True, stop=True)
                        nc.scalar.copy(out=r2t[:, j:j + 1], in_=cp[:])

                    fp3 = psum_d.tile([1, 40], F32, tag="fc")
                    for j in range(2):
                        nc.tensor.matmul(out=fp3[:], lhsT=r2t[:, j:j + 1],
                                         rhs=f3r[:, j * 40:(j + 1) * 40],
                                         start=(j == 0), stop=False)
                    nc.tensor.matmul(out=fp3[:], lhsT=ones[:1, :1], rhs=b8r[:1, :],
                                     start=False, stop=True)
                    outsb = c5pool.tile([P, 40], F32, tag="outsb")
                    nc.scalar.copy(out=outsb[:1, :], in_=fp3[:])
                    nc.sync.dma_start(out=out_d[:, :], in_=outsb[:1, :])

            if rep_cm is not None:
                rep_cm.__exit__(None, None, None)

    nc.compile()
    return nc


def _prep_all(inputs):
    f32 = np.float32
    g = lambda k: np.asarray(inputs[k], dtype=f32)
    common = {}
    for i, (C, O) in enumerate(LAYERS, start=1):
        w, s, b = g(f"w{i}"), g(f"s{i}"), g(f"b{i}")
        wa = w[:, :C] * s[:, None]
        wb = w[:, C:] * s[:, None]
        common[f"wg{i}"] = np.ascontiguousarray(wa.T)
        common[f"wc{i}"] = np.ascontiguousarray((wb - wa).T)
        common[f"bb{i}"] = np.ascontiguousarray(b[None, :])
    # conv5: chunk rows by xcat segments [x1(64), x2(64), x3(128), x4a, x4b]
    W5p = np.ascontiguousarray((g("w5") * g("s5")[:, None]).T)  # (512, 1024)
    w5r = np.zeros((P, 5 * 1024), f32)
    segs = [(0, 64), (64, 64), (128, 128), (256, 128), (384, 128)]
    for blk, (r0, k) in enumerate(segs):
        w5r[:k, blk * 1024:(blk + 1) * 1024] = W5p[r0:r0 + k]
    common["w5r"] = w5r
    common["b5r"] = np.ascontiguousarray(g("b5")[None, :])
    F1 = np.ascontiguousarray((g("fc1_w") * g("s6")[:, None]).T)  # (2048, 512)
    F1[1024:, :] *= f32(1.0 / N)
    common["f1r"] = np.concatenate([F1[128 * j:128 * (j + 1)] for j in range(16)],
                                   axis=1)
    common["b6r"] = np.ascontiguousarray(g("b6")[None, :])
    F2 = np.ascontiguousarray((g("fc2_w") * g("s7")[:, None]).T)  # (512, 256)
    common["f2r"] = np.concatenate([F2[128 * j:128 * (j + 1)] for j in range(4)],
                                   axis=1)
    common["b7r"] = np.ascontiguousarray(g("b7")[None, :])
    F3 = np.ascontiguousarray(g("fc3_w").T)  # (256, 40)
    common["f3r"] = np.concatenate([F3[128 * j:128 * (j + 1)] for j in range(2)],
                                   axis=1)
    common["b8r"] = np.ascontiguousarray(g("fc3_b")[None, :])
    x = np.asarray(inputs["x"], dtype=f32)  # (8, 3, 2048)
    in_maps = [dict(common, x=np.ascontiguousarray(x[c])) for c in range(NCORES)]
    return in_maps


_NC_CACHE = None


def _get_nc():
    global _NC_CACHE
    if _NC_CACHE is None:
        _NC_CACHE = build_program()
    return _NC_CACHE


def kernel(**inputs):
    nc = _get_nc()
    in_maps = _prep_all(inputs)
    res = run_bass_kernel_spmd(nc, in_maps, core_ids=list(range(NCORES)))
    out = np.stack([res.results[c]["out"][0] for c in range(NCORES)], axis=0)
    return out.astype(np.float32)


# revision 21
# speedup vs baseline: 1.0757x; 1.0757x over previous
"""DGCNN (nn_DGCNN_Model) Trainium2 Bass kernel.

Pure data-parallel over batch B=8 across 8 NeuronCores (one sample per
core, SPMD — identical program, per-core inputs).

Per-sample pipeline (N=2048 points, K=20 neighbors):
  Each EdgeConv(C->O) is decomposed as
     h[n,k,o] = G[idx[n,k],o] + C2[n,o]
     out[n,o] = lrelu( max_k h[n,k,o] )          (BN scale folded, s>0)
  with G = X @ (s*w_a)^T  (N,O),  C2 = X @ (s*(w_b-w_a))^T + b  (N,O).
  kNN scores S[n,j] = 2<x_n,x_j> - |x_j|^2 (row-constant -|x_n|^2 dropped;
  it does not change per-row top-k). Top-20 per row via DVE max8 /
  max_index / match_replace (3 rounds of 8). Neighbor rows of G gathered
  from a DRAM table via indirect DMA (one per neighbor slot), then
  max-reduced on DVE.
"""
import sys

sys.path.insert(0, "/opt/trn_rl_repo")

import numpy as np

import concourse.bass as bass
import concourse.bacc as bacc
import concourse.mybir as mybir
import concourse.tile as tile
from concourse.bass import IndirectOffsetOnAxis
from concourse.bass_utils import run_bass_kernel_spmd
from concourse.masks import make_identity

P = 128
N = 2048
NT = N // P  # 16 row tiles
KNB = 20
NCORES = 8
F32 = mybir.dt.float32
U32 = mybir.dt.uint32
NEG_BIG = -1.0e30

# (C_in, O_out) for the four EdgeConvs
LAYERS = [(3, 64), (64, 64), (64, 128), (128, 256)]


def _emit_edge_conv(nc, pools, C, O, x_ap, wg, wc, bb, g_dram, write_out):
    """Emit one EdgeConv layer.

    x_ap:  AP (C, N) — input features, feature-major, base partition 0.
    wg/wc: AP (C, O) SBUF — projection weights.   bb: AP (1, O) — bias row.
    g_dram: DRAM tensor handle (N, O) — scratch table for gathers.
    write_out(t, y): store tile-t output (P, O) into next-layer features.
    """
    ones, neg1 = pools["ones"], pools["neg1"]
    psum_s, psum_ab = pools["psum_s"], pools["psum_ab"]
    pool = pools["sbuf"]

    # ---- phase A: squared norms + projections ----
    # chunked (512-col) so each chunk only depends on the producing tiles
    # of the previous layer -> cross-layer overlap.
    xsq = pool.tile([P, N], F32, tag="xa")
    x2 = pool.tile([P, N], F32, tag="xa")
    xx_sb = pool.tile([1, N], F32, tag="xx", bufs=1)
    for j in range(4):
        cs = slice(j * 512, (j + 1) * 512)
        nc.scalar.activation(out=xsq[:C, cs], in_=x_ap[:, cs],
                             func=mybir.ActivationFunctionType.Square)
        xx_ps = psum_ab.tile([1, 512], F32, tag="ab", name=f"xxps{j}")
        nc.tensor.matmul(out=xx_ps[:], lhsT=ones[:C, :1], rhs=xsq[:C, cs],
                         start=True, stop=True)
        nc.scalar.copy(out=xx_sb[:, cs], in_=xx_ps[:])
        nc.scalar.activation(out=x2[:C, cs], in_=x_ap[:, cs],
                             func=mybir.ActivationFunctionType.Copy, scale=2.0)

    c2full = pool.tile([P, NT * O], F32, tag="c2full", bufs=1)
    for t in range(NT):
        rs = slice(t * P, (t + 1) * P)
        os_ = slice(t * O, (t + 1) * O)
        gp = psum_ab.tile([P, O], F32, tag="ab")
        nc.tensor.matmul(out=gp[:], lhsT=x_ap[:, rs], rhs=wg, start=True, stop=True)
        gsb = pool.tile([P, O], F32, tag="gsb")
        nc.scalar.copy(out=gsb[:], in_=gp[:])
        nc.sync.dma_start(out=g_dram[t * P:(t + 1) * P, :], in_=gsb[:])
        cp = psum_ab.tile([P, O], F32, tag="ab")
        nc.tensor.matmul(out=cp[:], lhsT=x_ap[:, rs], rhs=wc, start=True, stop=False)
        nc.tensor.matmul(out=cp[:], lhsT=ones[:1, :], rhs=bb, start=False, stop=True)
        nc.scalar.copy(out=c2full[:, os_], in_=cp[:])

    # ---- phase B: per-tile knn + gather-max + epilogue ----
    def _emit_epilogue(t, tg):
        m = pool.tile([P, O], F32, tag="m")
        nc.vector.tensor_reduce(out=m[:], in_=tg[:].rearrange("p k o -> p o k"),
                                axis=mybir.AxisListType.X, op=mybir.AluOpType.max)
        t1 = pool.tile([P, O], F32, tag="t1")
        nc.vector.tensor_tensor(out=t1[:], in0=m[:], in1=c2full[:, t * O:(t + 1) * O],
                                op=mybir.AluOpType.add)
        y = pool.tile([P, O], F32, tag="y")
        nc.vector.scalar_tensor_tensor(out=y[:], in0=t1[:], scalar=0.2, in1=t1[:],
                                       op0=mybir.AluOpType.mult,
                                       op1=mybir.AluOpType.max)
        write_out(t, y)

    pending = []
    for t in range(NT):
        rs = slice(t * P, (t + 1) * P)
        # scores in two PSUM halves so the copy frees banks earlier
        sph = [psum_s.tile([P, N // 2], F32, tag="s", name=f"sp{t}_{h}")
               for h in range(2)]
        ssb = pool.tile([P, N], F32, tag="ssb", bufs=3)
        for j in range(4):
            h, sp = j // 2, sph[j // 2]
            cs = slice(j * 512, (j + 1) * 512)
            cl = slice((j % 2) * 512, (j % 2 + 1) * 512)
            nc.tensor.matmul(out=sp[:, cl], lhsT=x2[:C, rs], rhs=x_ap[:, cs],
                             start=True, stop=False)
            nc.tensor.matmul(out=sp[:, cl], lhsT=neg1[:1, :], rhs=xx_sb[:, cs],
                             start=False, stop=True)
        for h in range(2):
            nc.scalar.copy(out=ssb[:, h * (N // 2):(h + 1) * (N // 2)],
                           in_=sph[h][:])

        # topk rounds; gathers for each round issue as soon as its indices
        # exist (separate index tiles -> independent dependencies)
        vals = pool.tile([P, 24], F32, tag="vals")
        idxsA = pool.tile([P, 8], U32, tag="idxsA")
        idxsB = pool.tile([P, 8], U32, tag="idxsB")
        idxsC = pool.tile([P, 8], U32, tag="idxsC")
        tg = pool.tile([P, KNB, O], F32, tag="tg", bufs=3)
        w1 = pool.tile([P, N], F32, tag="wrk")

        def gathers(idxt, k0, k1):
            for k in range(k0, k1):
                nc.gpsimd.indirect_dma_start(
                    out=tg[:, k, :], out_offset=None, in_=g_dram[:, :],
                    in_offset=IndirectOffsetOnAxis(ap=idxt[:, k - k0:k - k0 + 1],
                                                   axis=0),
                )

        nc.vector.max(out=vals[:, 0:8], in_=ssb[:])
        nc.vector.max_index(out=idxsA[:], in_max=vals[:, 0:8], in_values=ssb[:])
        gathers(idxsA, 0, 8)
        nc.vector.match_replace(out=w1[:], in_to_replace=vals[:, 0:8],
                                in_values=ssb[:], imm_value=NEG_BIG)
        nc.vector.max(out=vals[:, 8:16], in_=w1[:])
        nc.vector.max_index(out=idxsB[:], in_max=vals[:, 8:16], in_values=w1[:])
        gathers(idxsB, 8, 16)
        nc.vector.match_replace(out=ssb[:], in_to_replace=vals[:, 8:16],
                                in_values=w1[:], imm_value=NEG_BIG)
        nc.vector.max(out=vals[:, 16:24], in_=ssb[:])
        nc.vector.max_index(out=idxsC[:], in_max=vals[:, 16:24], in_values=ssb[:])
        gathers(idxsC, 16, KNB)

        # software pipelining: the epilogue of the PREVIOUS tile is emitted
        # here so its gather-dependent reduce sits after this tile's topk in
        # the serial DVE stream (engines execute in order — emitting the
        # reduce right after its own gathers would stall DVE on the DMAs).
        if len(pending) == 2:
            _emit_epilogue(*pending.pop(0))
        pending.append((t, tg))
    for args in pending:
        _emit_epilogue(*args)



def build_program(repeat=None):
    nc = bacc.Bacc("TRN2", num_devices=NCORES, debug=False)

    # ---------------- I/O declarations ----------------
    x_in = nc.dram_tensor("x", (3, N), F32, kind="ExternalInput")
    wgt = {}
    for i, (C, O) in enumerate(LAYERS, start=1):
        wgt[f"wg{i}"] = nc.dram_tensor(f"wg{i}", (C, O), F32, kind="ExternalInput")
        wgt[f"wc{i}"] = nc.dram_tensor(f"wc{i}", (C, O), F32, kind="ExternalInput")
        wgt[f"bb{i}"] = nc.dram_tensor(f"bb{i}", (1, O), F32, kind="ExternalInput")
    w5r_d = nc.dram_tensor("w5r", (P, 5 * 1024), F32, kind="ExternalInput")
    b5r_d = nc.dram_tensor("b5r", (1, 1024), F32, kind="ExternalInput")
    f1r_d = nc.dram_tensor("f1r", (P, 16 * 512), F32, kind="ExternalInput")
    b6r_d = nc.dram_tensor("b6r", (1, 512), F32, kind="ExternalInput")
    f2r_d = nc.dram_tensor("f2r", (P, 4 * 256), F32, kind="ExternalInput")
    b7r_d = nc.dram_tensor("b7r", (1, 256), F32, kind="ExternalInput")
    f3r_d = nc.dram_tensor("f3r", (P, 2 * 40), F32, kind="ExternalInput")
    b8r_d = nc.dram_tensor("b8r", (1, 40), F32, kind="ExternalInput")
    out_d = nc.dram_tensor("out", (1, 40), F32, kind="ExternalOutput")

    g_drams = [nc.dram_tensor(f"gtab{i}", (N, O), F32)
               for i, (C, O) in enumerate(LAYERS, start=1)]

    with tile.TileContext(nc) as tc:
        with tc.tile_pool(name="glob", bufs=1) as gpool:
            # constants (full 128-partition tiles so base partitions align)
            ones = gpool.tile([P, P], F32, tag="ones")
            nc.vector.memset(ones[:], 1.0)
            neg1 = gpool.tile([P, P], F32, tag="neg1")
            nc.vector.memset(neg1[:], -1.0)
            identity = gpool.tile([P, P], F32, tag="identity")
            make_identity(nc, identity[:])

            # persistent transposed feature tensors (x1, x2, x3, x4a, x4b)
            TA = gpool.tile([P, N], F32, tag="TA")
            TB = gpool.tile([P, N], F32, tag="TB")
            T1 = gpool.tile([P, N], F32, tag="T1")
            T2 = gpool.tile([P, N], F32, tag="T2")
            T3 = gpool.tile([P, N], F32, tag="T3")

            # small per-layer weights (always 128-partition allocs)
            wsb = {}
            for i, (C, O) in enumerate(LAYERS, start=1):
                for nm, rows, cols in ((f"wg{i}", C, O), (f"wc{i}", C, O),
                                       (f"bb{i}", 1, O)):
                    tl = gpool.tile([P, cols], F32, tag=nm)
                    nc.sync.dma_start(out=tl[:rows, :], in_=wgt[nm][:, :])
                    wsb[nm] = tl
            x1sb = gpool.tile([P, N], F32, tag="x1sb")
            nc.sync.dma_start(out=x1sb[:3, :], in_=x_in[:, :])

            rep_cm = tc.For_i(0, repeat, 1) if repeat else None
            if rep_cm is not None:
                rep_cm.__enter__()

            # -------- edge conv layers --------
            with (
                tc.tile_pool(name="lay_sbuf", bufs=2) as lpool,
                tc.tile_pool(name="lay_psum_s", bufs=1, space="PSUM") as psum_s,
                tc.tile_pool(name="lay_psum_ab", bufs=2, space="PSUM") as psum_ab,
                tc.tile_pool(name="lay_psum_tr", bufs=2, space="PSUM") as psum_tr,
            ):
                pools = {"ones": ones, "neg1": neg1, "sbuf": lpool,
                         "psum_s": psum_s, "psum_ab": psum_ab}

                # destination map: layer idx -> list of (col0, ncols, T)
                outmaps = [
                    [(0, 64, TA)],
                    [(0, 64, TB)],
                    [(0, 128, T1)],
                    [(0, 128, T2), (128, 128, T3)],
                ]
                xin_aps = [x1sb[:3, :], TA[:64, :], TB[:64, :], T1[:, :]]

                for li, (C, O) in enumerate(LAYERS):
                    def write_out(t, y, _li=li):
                        for (c0, nco, Tdst) in outmaps[_li]:
                            tp = psum_tr.tile([P, P], F32, tag="tr")
                            nc.tensor.transpose(out=tp[:nco, :],
                                                in_=y[:, c0:c0 + nco],
                                                identity=identity[:])
                            nc.scalar.copy(out=Tdst[:nco, t * P:(t + 1) * P],
                                           in_=tp[:nco, :])
                    _emit_edge_conv(nc, pools, C, O, xin_aps[li],
                                    wsb[f"wg{li+1}"][:C, :], wsb[f"wc{li+1}"][:C, :],
                                    wsb[f"bb{li+1}"][:1, :], g_drams[li], write_out)

            # -------- conv5 + global pooling --------
            with (
                tc.tile_pool(name="c5_sbuf", bufs=2) as c5pool,
                tc.tile_pool(name="c5_psum_mean", bufs=1, space="PSUM") as psum_mean,
            ):
                # conv5 K-chunks: (features, w5r block, K)
                chunks = [(TA, 0, 64), (TB, 1, 64), (T1, 2, 128),
                          (T2, 3, 128), (T3, 4, 128)]
                w5r = c5pool.tile([P, 5 * 1024], F32, tag="w5r", bufs=1)
                nc.sync.dma_start(out=w5r[:], in_=w5r_d[:, :])
                b5r = c5pool.tile([P, 1024], F32, tag="b5r", bufs=1)
                nc.sync.dma_start(out=b5r[:1, :], in_=b5r_d[:, :])
                HM = c5pool.tile([P, 1024], F32, tag="HM", bufs=1)
                mean_ps = psum_mean.tile([1, 1024], F32, tag="mean")
                psum5_cm = tc.tile_pool(name="c5_psum", bufs=2, space="PSUM")
                psum5 = psum5_cm.__enter__()
                for t in range(NT):
                    rs = slice(t * P, (t + 1) * P)
                    hp = psum5.tile([P, 1024], F32, tag="hp")
                    for ci, (Tt, blk, kc) in enumerate(chunks):
                        for h in range(2):
                            ns = slice(h * 512, (h + 1) * 512)
                            nc.tensor.matmul(
                                out=hp[:, ns], lhsT=Tt[:kc, rs],
                                rhs=w5r[:kc, blk * 1024 + h * 512:
                                        blk * 1024 + (h + 1) * 512],
                                start=(ci == 0), stop=False)
                    for h in range(2):
                        ns = slice(h * 512, (h + 1) * 512)
                        nc.tensor.matmul(out=hp[:, ns], lhsT=ones[:1, :],
                                         rhs=b5r[:1, ns], start=False, stop=True)
                    hc = c5pool.tile([P, 1024], F32, tag="hc")
                    nc.scalar.copy(out=hc[:], in_=hp[:])
                    hs = c5pool.tile([P, 1024], F32, tag="hs")
                    nc.vector.scalar_tensor_tensor(
                        out=hs[:], in0=hc[:], scalar=0.2, in1=hc[:],
                        op0=mybir.AluOpType.mult, op1=mybir.AluOpType.max)
                    if t == 0:
                        nc.vector.tensor_copy(out=HM[:], in_=hs[:])
                    else:
                        nc.vector.tensor_tensor(out=HM[:], in0=HM[:], in1=hs[:],
                                                op=mybir.AluOpType.max)
                    for h in range(2):
                        ns = slice(h * 512, (h + 1) * 512)
                        nc.tensor.matmul(out=mean_ps[:, ns], lhsT=ones[:, :1],
                                         rhs=hs[:, ns], start=(t == 0),
                                         stop=(t == NT - 1))

                psum5_cm.__exit__(None, None, None)
                with tc.tile_pool(name="d_psum", bufs=2, space="PSUM") as psum_d:
                    mp = c5pool.tile([P, 8], F32, tag="mp")
                    mn = c5pool.tile([P, 8], F32, tag="mn")
                    for j in range(8):
                        tp = psum_d.tile([P, P], F32, tag="dtr")
                        nc.tensor.transpose(out=tp[:], in_=HM[:, j * P:(j + 1) * P],
                                            identity=identity[:])
                        nc.vector.tensor_reduce(out=mp[:, j:j + 1], in_=tp[:],
                                                axis=mybir.AxisListType.X,
                                                op=mybir.AluOpType.max)
                    mean_sb = c5pool.tile([P, 1024], F32, tag="mean_sb")
                    nc.scalar.copy(out=mean_sb[:1, :], in_=mean_ps[:])
                    for j in range(8):
                        cp = psum_d.tile([P, 1], F32, tag="dcol")
                        nc.tensor.matmul(out=cp[:], lhsT=mean_sb[:1, j * P:(j + 1) * P],
                                         rhs=ones[:1, :1], start=True, stop=True)
                        nc.scalar.copy(out=mn[:, j:j + 1], in_=cp[:])

                    # -------- FC head --------
                    f1r = c5pool.tile([P, 16 * 512], F32, tag="f1r", bufs=1)
                    nc.sync.dma_start(out=f1r[:], in_=f1r_d[:, :])
                    b6r = c5pool.tile([P, 512], F32, tag="b6r")
                    nc.sync.dma_start(out=b6r[:1, :], in_=b6r_d[:, :])
                    f2r = c5pool.tile([P, 4 * 256], F32, tag="f2r")
                    nc.sync.dma_start(out=f2r[:], in_=f2r_d[:, :])
                    b7r = c5pool.tile([P, 256], F32, tag="b7r")
                    nc.sync.dma_start(out=b7r[:1, :], in_=b7r_d[:, :])
                    f3r = c5pool.tile([P, 2 * 40], F32, tag="f3r")
                    nc.sync.dma_start(out=f3r[:], in_=f3r_d[:, :])
                    b8r = c5pool.tile([P, 40], F32, tag="b8r")
                    nc.sync.dma_start(out=b8r[:1, :], in_=b8r_d[:, :])

                    fp1 = psum_d.tile([1, 512], F32, tag="fc")
                    for j in range(8):
                        nc.tensor.matmul(out=fp1[:], lhsT=mp[:, j:j + 1],
                                         rhs=f1r[:, j * 512:(j + 1) * 512],
                                         start=(j == 0), stop=False)
                    for j in range(8):
                        nc.tensor.matmul(out=fp1[:], lhsT=mn[:, j:j + 1],
                                         rhs=f1r[:, (8 + j) * 512:(9 + j) * 512],
                                         start=False, stop=False)
                    nc.tensor.matmul(out=fp1[:], lhsT=ones[:1, :1], rhs=b6r[:1, :],
                                     start=False, stop=True)
                    r1 = c5pool.tile([P, 512], F32, tag="r1")
                    nc.scalar.activation(out=r1[:1, :], in_=fp1[:],
                                         func=mybir.ActivationFunctionType.Relu)
                    r1t = c5pool.tile([P, 4], F32, tag="r1t")
                    for j in range(4):
                        cp = psum_d.tile([P, 1], F32, tag="dcol")
                        nc.tensor.matmul(out=cp[:], lhsT=r1[:1, j * P:(j + 1) * P],
                                         rhs=ones[:1, :1], start=True, stop=True)
                        nc.scalar.copy(out=r1t[:, j:j + 1], in_=cp[:])

                    fp2 = psum_d.tile([1, 256], F32, tag="fc")
                    for j in range(4):
                        nc.tensor.matmul(out=fp2[:], lhsT=r1t[:, j:j + 1],
                                         rhs=f2r[:, j * 256:(j + 1) * 256],
                                         start=(j == 0), stop=False)
                    nc.tensor.matmul(out=fp2[:], lhsT=ones[:1, :1], rhs=b7r[:1, :],
                                     start=False, stop=True)
                    r2 = c5pool.tile([P, 256], F32, tag="r2")
                    nc.scalar.activation(out=r2[:1, :], in_=fp2[:],
                                         func=mybir.ActivationFunctionType.Relu)
                    r2t = c5pool.tile([P, 2], F32, tag="r2t")
                    for j in range(2):
                        cp = psum_d.tile([P, 1], F32, tag="dcol")
                        nc.tensor.matmul(out=cp[:], lhsT=r2[:1, j * P:(j + 1) * P],
                                         rhs=ones[:1, :1], start=True, stop=True)
                        nc.scalar.copy(out=r2t[:, j:j + 1], in_=cp[:])

                    fp3 = psum_d.tile([1, 40], F32, tag="fc")
                    for j in range(2):
                        nc.tensor.matmul(out=fp3[:], lhsT=r2t[:, j:j + 1],
                                         rhs=f3r[:, j * 40:(j + 1) * 40],
                                         start=(j == 0), stop=False)
                    nc.tensor.matmul(out=fp3[:], lhsT=ones[:1, :1], rhs=b8r[:1, :],
                                     start=False, stop=True)
                    outsb = c5pool.tile([P, 40], F32, tag="outsb")
                    nc.scalar.copy(out=outsb[:1, :], in_=fp3[:])
                    nc.sync.dma_start(out=out_d[:, :], in_=outsb[:1, :])

            if rep_cm is not None:
                rep_cm.__exit__(None, None, None)

    nc.compile()
    return nc


def _prep_all(inputs):
    f32 = np.float32
    g = lambda k: np.asarray(inputs[k], dtype=f32)
    common = {}
    for i, (C, O) in enumerate(LAYERS, start=1):
        w, s, b = g(f"w{i}"), g(f"s{i}"), g(f"b{i}")
        wa = w[:, :C] * s[:, None]
        wb = w[:, C:] * s[:, None]
        common[f"wg{i}"] = np.ascontiguousarray(wa.T)
        common[f"wc{i}"] = np.ascontiguousarray((wb - wa).T)
        common[f"bb{i}"] = np.ascontiguousarray(b[None, :])
    # conv5: chunk rows by xcat segments [x1(64), x2(64), x3(128), x4a, x4b]
    W5p = np.ascontiguousarray((g("w5") * g("s5")[:, None]).T)  # (512, 1024)
    w5r = np.zeros((P, 5 * 1024), f32)
    segs = [(0, 64), (64, 64), (128, 128), (256, 128), (384, 128)]
    for blk, (r0, k) in enumerate(segs):
        w5r[:k, blk * 1024:(blk + 1) * 1024] = W5p[r0:r0 + k]
    common["w5r"] = w5r
    common["b5r"] = np.ascontiguousarray(g("b5")[None, :])
    F1 = np.ascontiguousarray((g("fc1_w") * g("s6")[:, None]).T)  # (2048, 512)
    F1[1024:, :] *= f32(1.0 / N)
    common["f1r"] = np.concatenate([F1[128 * j:128 * (j + 1)] for j in range(16)],
                                   axis=1)
    common["b6r"] = np.ascontiguousarray(g("b6")[None, :])
    F2 = np.ascontiguousarray((g("fc2_w") * g("s7")[:, None]).T)  # (512, 256)
    common["f2r"] = np.concatenate([F2[128 * j:128 * (j + 1)] for j in range(4)],
                                   axis=1)
    common["b7r"] = np.ascontiguousarray(g("b7")[None, :])
    F3 = np.ascontiguousarray(g("fc3_w").T)  # (256, 40)
    common["f3r"] = np.concatenate([F3[128 * j:128 * (j + 1)] for j in range(2)],
                                   axis=1)
    common["b8r"] = np.ascontiguousarray(g("fc3_b")[None, :])
    x = np.asarray(inputs["x"], dtype=f32)  # (8, 3, 2048)
    in_maps = [dict(common, x=np.ascontiguousarray(x[c])) for c in range(NCORES)]
    return in_maps


_NC_CACHE = None


def _get_nc():
    global _NC_CACHE
    if _NC_CACHE is None:
        _NC_CACHE = build_program()
    return _NC_CACHE


def kernel(**inputs):
    nc = _get_nc()
    in_maps = _prep_all(inputs)
    res = run_bass_kernel_spmd(nc, in_maps, core_ids=list(range(NCORES)))
    out = np.stack([res.results[c]["out"][0] for c in range(NCORES)], axis=0)
    return out.astype(np.float32)
